# revision 1
# baseline (speedup 1.0000x reference)
"""Trainium2 Bass kernel for CRF negative log-likelihood (nn_CRF).

Math (reference semantics, tags always valid in [0,128)):
  nll = -mean_b(scores[b] - log_z[b]) / 100

  scores[b] = em[b,0,tag_0] + T[BOS,tag_0] + sum_{s>=1}(em[b,s,tag_s] + T[tag_{s-1},tag_s])
              + T[tag_last, EOS]
  log_z[b]  = forward-algorithm partition function over the 128 real labels
              (BOS/EOS rows/cols are exactly unreachable: exp(-10000)=0 in fp32).

Device strategy (8 cores x 2 chains = 16 sequence chunks of 128 steps):
  * Forward recursion in the exp domain: q <- (q @ expT) * exp(em_s - K) with
    constant per-step rescale exp(-K). Each chunk starts from a uniform vector
    with W=8 warmup steps; the random dense CRF forward map contracts to ~1e-8
    within 8 steps (validated numerically), so each chunk's log-gain is exact.
    Chunk gains telescope:
      log_z = phi_end(chunk0) + sum_{others}(phi_end - phi_pre) + 2047*K,
    phi = log(f . q), f = ones (exp(T[:,EOS]) at the sequence end). Chunk 0
    gets an exact initial state u0 = exp(em_0 + T[BOS,:]) blended in via a
    data-driven gamma scalar. Two chains per core pipeline each other's
    PE->PSUM->DVE latency, letting the per-step DVE multiply run at full
    width [128, 256] (one op per step per chain).
  * Gold-path score via a PE diag-accumulate stream: one-hot masks M_s[l,b]
    (fp8) as stationary weights against rhs = [em_s | T_col_{s+1}]
    (T_col_s[i,b] = T[i, tag_s(b)]); PSUM accumulates over all steps
       dacc_h[b',0:128]   += sum_l M_s[l,b'] em_s[l,b]      (emission score)
       dacc_h[b',128:256] += sum_l M_s[l,b'] T_col[l,b]     (transition score)
    whose diagonals are the per-batch score sums. BOS term rides in a
    repurposed warmup slot; the EOS term is the T_col slice one past the end.

The program is fully SPMD: all per-core differences are carried by input
data (zero-padded warmup slices, gamma blend scalars, BOS bias vectors,
final functional vector).
"""
import sys, os

for _p in ("/opt/trn_rl_repo",):
    if _p not in sys.path and os.path.isdir(_p):
        sys.path.insert(0, _p)

import numpy as np
import ml_dtypes

B, S, NL = 256, 2048, 128
NB, BOS, EOS = 130, 128, 129
NCORES = 8
NCHAIN = 2             # chains per core
CSTEP = 128            # real steps per chain
W = 8                  # warmup slots per chain
TILES = W + CSTEP      # 136 slots per chain
CHUNK = 8              # slots per DMA/exp chunk
NCH = TILES // CHUNK   # 17 chunks per chain
F8 = ml_dtypes.float8_e4m3
BF16 = ml_dtypes.bfloat16

_prog_cache = {}


def _estimate_K(em, T):
    """Mean per-step log-growth of the forward recursion (host, tiny presim)."""
    expT = np.exp(T[:NL, :NL].astype(np.float64))
    nb = 4
    v = np.exp(T[BOS, :NL].astype(np.float64)[None, :] + em[:nb, 0, :].astype(np.float64))
    g = []
    for s in range(1, 33):
        v = (v @ expT) * np.exp(em[:nb, s, :].astype(np.float64))
        n = v.sum(axis=1)
        g.append(np.log(n))
        v /= n[:, None]
    g = np.array(g[8:])  # skip mixing transient
    return float(g.mean())


def _group(a):
    """[TILES, NL, X] -> [NCH, NL, CHUNK*X] chunk-grouped, DMA-friendly."""
    t, nl, x = a.shape
    return np.ascontiguousarray(
        a.reshape(t // CHUNK, CHUNK, nl, x).transpose(0, 2, 1, 3)
    ).reshape(t // CHUNK, nl, CHUNK * x)


def _host_prep(emissions, tags, transitions):
    em = np.asarray(emissions, np.float32)
    tg = np.asarray(tags, np.int64)
    T = np.asarray(transitions, np.float32)

    K = _estimate_K(em, T)
    expT_bf = np.exp(T[:NL, :NL]).astype(BF16)            # [prev, cur]
    teos_bf = np.exp(T[:NL, EOS]).astype(BF16)
    T8 = T[:NL, :NL].astype(F8)

    em_t = np.ascontiguousarray(em.transpose(1, 2, 0)).astype(F8)     # [S, 128, B]
    M = np.zeros((S, NL, B), F8)
    M[np.arange(S)[:, None], tg.T, np.arange(B)[None, :]] = 1.0
    T_col = np.ascontiguousarray(np.ascontiguousarray(T8[:, tg.T]).transpose(1, 0, 2))  # [S,128,B]

    tbos_row_f8 = np.broadcast_to(T[BOS, :NL].astype(F8)[:, None], (NL, B))
    teos_col_f8 = np.broadcast_to(T[:NL, EOS].astype(F8)[:, None], (NL, B))

    in_maps = []
    for k in range(NCORES):
        emt = np.zeros((NCHAIN, TILES, NL, B), F8)
        dmask = np.zeros((NCHAIN, TILES, NL, B), F8)
        dstr = np.zeros((NCHAIN, TILES, NL, 2 * B), F8)
        tbos = np.full((NL, NCHAIN), -10000.0, np.float32)
        gam = np.ones((NL, NCHAIN), np.float32)
        for ch in range(NCHAIN):
            s0 = CSTEP * (NCHAIN * k + ch)
            lo = s0 - W
            for j in range(TILES):
                s = lo + j
                if s >= 0:
                    emt[ch, j] = em_t[s]
                if j >= W:
                    dmask[ch, j] = M[s]
                    dstr[ch, j, :, 0:NL] = em_t[s][:, 0:NL]
                    dstr[ch, j, :, 2 * NL:3 * NL] = em_t[s][:, NL:B]
                    tc = T_col[s + 1] if s + 1 < S else teos_col_f8
                    dstr[ch, j, :, NL:2 * NL] = tc[:, 0:NL]
                    dstr[ch, j, :, 3 * NL:4 * NL] = tc[:, NL:B]
            if k == 0 and ch == 0:
                # BOS term in repurposed warmup slot: diag(M0^T TBrow) = T[BOS, tag0]
                dmask[0, W - 1] = M[0]
                dstr[0, W - 1, :, 0:NL] = 0.0
                dstr[0, W - 1, :, 2 * NL:3 * NL] = 0.0
                dstr[0, W - 1, :, NL:2 * NL] = tbos_row_f8[:, 0:NL]
                dstr[0, W - 1, :, 3 * NL:4 * NL] = tbos_row_f8[:, NL:B]
                tbos[:, 0] = T[BOS, :NL]
                gam[:, 0] = 0.0

        fvec = (teos_bf if k == NCORES - 1 else np.ones(NL, BF16))[:, None]

        ga = [_group(emt[c]) for c in range(NCHAIN)]
        gm = [_group(dmask[c]) for c in range(NCHAIN)]
        gd = [_group(dstr[c]) for c in range(NCHAIN)]
        # stream A: [NCH, NL, 2*CB] = em(ch0)|em(ch1)
        sA = np.concatenate(ga, axis=2)
        # stream B: [NCH, NL, 2*CB + 2*2CB] = dmask(ch0)|dmask(ch1)|dstr(ch0)|dstr(ch1)
        sB = np.concatenate(gm + gd, axis=2)
        cb = np.zeros((NL, 2 * NL + 2), BF16)
        cb[:, 0:NL] = expT_bf
        cb[:, NL:2 * NL] = np.eye(NL, dtype=BF16)
        cb[:, 2 * NL:2 * NL + 1] = np.ones((NL, 1), BF16)
        cb[:, 2 * NL + 1:2 * NL + 2] = fvec
        cf = np.zeros((NL, 2 * NCHAIN), np.float32)
        cf[:, 0:NCHAIN] = tbos
        cf[:, NCHAIN:2 * NCHAIN] = gam
        in_maps.append({"sa": np.ascontiguousarray(sA), "sb": np.ascontiguousarray(sB),
                        "cbf": cb, "cfp": cf})
    return in_maps, K


def _build_program(K):
    import contextlib
    import concourse.bass as bass
    import concourse.tile as tile
    from concourse import bacc, mybir

    dt = mybir.dt
    Alu = mybir.AluOpType
    Act = mybir.ActivationFunctionType

    nc = bacc.Bacc("TRN2", target_bir_lowering=False, debug=False, num_devices=NCORES)

    CB = CHUNK * B
    sa_d = nc.dram_tensor("sa", [NCH, NL, 2 * CB], dt.float8e4, kind="ExternalInput").ap()
    sb_d = nc.dram_tensor("sb", [NCH, NL, 6 * CB], dt.float8e4, kind="ExternalInput").ap()
    cbf_d = nc.dram_tensor("cbf", [NL, 2 * NL + 2], dt.bfloat16, kind="ExternalInput").ap()
    cfp_d = nc.dram_tensor("cfp", [NL, 2 * NCHAIN], dt.float32, kind="ExternalInput").ap()

    # per chain: [pre | post | end] each [1, 256]
    phis_d = nc.dram_tensor("phis", [1, NCHAIN * 3 * B], dt.float32, kind="ExternalOutput").ap()
    etpart_d = nc.dram_tensor("etpart", [NL, 4], dt.float32, kind="ExternalOutput").ap()

    with tile.TileContext(nc) as tc:
        with contextlib.ExitStack() as ctx:
            const = ctx.enter_context(tc.tile_pool(name="const", bufs=1))
            emring = ctx.enter_context(tc.tile_pool(name="emring", bufs=4))
            exring = ctx.enter_context(tc.tile_pool(name="exring", bufs=6))
            dring = ctx.enter_context(tc.tile_pool(name="dring", bufs=3))
            ps = ctx.enter_context(tc.tile_pool(name="ps", bufs=1, space="PSUM"))

            cbf = const.tile([NL, 2 * NL + 2], dt.bfloat16)
            nc.sync.dma_start(cbf[:], cbf_d[:])
            cfp = const.tile([NL, 2 * NCHAIN], dt.float32)
            nc.sync.dma_start(cfp[:], cfp_d[:])
            expT = cbf[:, 0:NL]
            ident = cbf[:, NL:2 * NL]
            fones = cbf[:, 2 * NL:2 * NL + 1]
            fvec = cbf[:, 2 * NL + 1:2 * NL + 2]
            tbos = cfp[:, 0:NCHAIN]
            gam = cfp[:, NCHAIN:2 * NCHAIN]
            negK = const.tile([NL, 1], dt.float32)
            nc.vector.memset(negK[:], -K)

            q0 = const.tile([NL, B], dt.bfloat16)
            nc.vector.memset(q0[:], 1.0)
            q1 = const.tile([NL, B], dt.bfloat16)
            nc.vector.memset(q1[:], 1.0)
            u0 = const.tile([NL, B], dt.bfloat16)
            u1 = const.tile([NL, B], dt.bfloat16)
            qs = (q0, q1)
            us = (u0, u1)

            ps0 = ps.tile([NL, B], dt.float32)
            ps1 = ps.tile([NL, B], dt.float32)
            daccA = ps.tile([NL, 2 * NL], dt.float32)
            daccB = ps.tile([NL, 2 * NL], dt.float32)
            phi_pp0 = ps.tile([1, 2 * B], dt.float32)   # chain0: [pre | post]
            phi_pp1 = ps.tile([1, 2 * B], dt.float32)   # chain1
            phi_end = ps.tile([1, NCHAIN * B], dt.float32)
            pss = (ps0, ps1)
            phis = (phi_pp0, phi_pp1)
            daccs = (daccA, daccB)

            exc = {}
            for c in range(NCH):
                a_t = emring.tile([NL, 2 * CB], dt.float8e4, name=f"sac{c}", tag="em")
                nc.sync.dma_start(a_t[:], sa_d[c])
                emc = {ch: a_t[:, ch * CB:(ch + 1) * CB] for ch in range(NCHAIN)}
                for ch in range(NCHAIN):
                    x_t = exring.tile([NL, CB], dt.bfloat16, name=f"exc{ch}_{c}", tag="ex")
                    nc.scalar.activation(x_t[:], emc[ch], Act.Exp, bias=negK[:], scale=1.0)
                    exc[ch] = x_t
                    if c == W // CHUNK:
                        nc.scalar.activation(us[ch][:], emc[ch][:, 0:B], Act.Exp,
                                             bias=tbos[:, ch:ch + 1], scale=1.0)
                b_t = dring.tile([NL, 6 * CB], dt.float8e4, name=f"sbc{c}", tag="d")
                nc.sync.dma_start(b_t[:], sb_d[c])
                mc = {ch: b_t[:, ch * CB:(ch + 1) * CB] for ch in range(NCHAIN)}
                dc = {ch: b_t[:, 2 * CB + ch * 2 * CB: 2 * CB + (ch + 1) * 2 * CB] for ch in range(NCHAIN)}

                for t8 in range(CHUNK):
                    t = c * CHUNK + t8
                    for ch in range(NCHAIN):
                        q, p = qs[ch], pss[ch]
                        if t == W:
                            nc.tensor.matmul(phis[ch][:, 0:B], fones[:], q[:],
                                             start=True, stop=True)
                        nc.tensor.matmul(p[:], expT[:], q[:], start=True, stop=True)
                        nc.vector.tensor_tensor(q[:], p[:], exc[ch][:, t8 * B:(t8 + 1) * B],
                                                Alu.mult)
                        if t == W:
                            nc.vector.scalar_tensor_tensor(q[:], q[:], gam[:, ch:ch + 1],
                                                           us[ch][:], Alu.mult, Alu.add)
                            nc.tensor.matmul(phis[ch][:, B:2 * B], fones[:], q[:],
                                             start=True, stop=True)
                        # diag accumulate: lhsT = dmask half, rhs = [em_h | tcol_h]
                        for g in range(2):
                            nc.tensor.matmul(
                                daccs[g][:],
                                mc[ch][:, t8 * B + g * NL: t8 * B + (g + 1) * NL],
                                dc[ch][:, t8 * 2 * B + g * 2 * NL: t8 * 2 * B + (g + 1) * 2 * NL],
                                start=(t == 0 and ch == 0), stop=(t == TILES - 1 and ch == NCHAIN - 1))

            for ch in range(NCHAIN):
                nc.tensor.matmul(phi_end[:, ch * B:(ch + 1) * B], fvec[:], qs[ch][:],
                                 start=True, stop=True)

            phi_sb = const.tile([1, NCHAIN * 3 * B], dt.float32)
            nc.scalar.copy(phi_sb[:, 0:2 * B], phi_pp0[:])
            nc.scalar.copy(phi_sb[:, 2 * B:4 * B], phi_pp1[:])
            nc.scalar.copy(phi_sb[:, 4 * B:6 * B], phi_end[:])
            nc.sync.dma_start(phis_d[:], phi_sb[:])

            escr = const.tile([NL, NL], dt.bfloat16)
            etp = const.tile([NL, 4], dt.float32)
            for g in range(2):
                nc.vector.scalar_tensor_tensor(escr[:], daccs[g][:, 0:NL], 1.0, ident[:],
                                               Alu.mult, Alu.mult, accum_out=etp[:, g:g + 1])
                nc.vector.scalar_tensor_tensor(escr[:], daccs[g][:, NL:2 * NL], 1.0, ident[:],
                                               Alu.mult, Alu.mult, accum_out=etp[:, 2 + g:3 + g])
            nc.sync.dma_start(etpart_d[:], etp[:])

    nc.compile()
    return nc


def run(emissions, tags, transitions, trace=False, trace_cores=None):
    from concourse.bass_utils import run_bass_kernel_spmd
    in_maps, K = _host_prep(emissions, tags, transitions)
    key = f"{K:.9f}"
    if key not in _prog_cache:
        _prog_cache[key] = _build_program(K)
    nc = _prog_cache[key]
    if trace:
        try:
            import axon_prof
            axon_prof.install()
        except Exception:
            pass
    r = run_bass_kernel_spmd(nc, in_maps, list(range(NCORES)), trace=trace,
                             trace_cores=trace_cores)

    # phis per core raw sums: [pre0|post0|pre1|post1|end0|end1] each [B]
    raw = np.stack([r.results[k]["phis"].reshape(6, B) for k in range(NCORES)]).astype(np.float64)
    raw = np.log(raw)
    phis = np.empty((NCORES * NCHAIN, 3, B))
    for k in range(NCORES):
        for ch in range(NCHAIN):
            phis[2 * k + ch, 0] = raw[k, 2 * ch + 0]      # pre
            phis[2 * k + ch, 1] = raw[k, 2 * ch + 1]      # post
            phis[2 * k + ch, 2] = raw[k, 4 + ch]          # end
    etp = np.stack([r.results[k]["etpart"] for k in range(NCORES)]).sum(0)  # [128, 4]

    log_z = phis[0, 2] + phis[1:, 2].sum(0) - phis[1:, 0].sum(0) + 2047.0 * K
    scores = (etp[:, 0:2] + etp[:, 2:4]).transpose(1, 0).reshape(2 * NL).astype(np.float64)
    nll = -np.mean(scores - log_z) / 100.0
    return np.float32(nll), r


def kernel(emissions, tags, transitions):
    out, _ = run(emissions, tags, transitions, trace=False)
    return out



# revision 3
# speedup vs baseline: 1.0082x; 1.0082x over previous
"""Trainium2 Bass kernel for CRF negative log-likelihood (nn_CRF).

Math (reference semantics, tags always valid in [0,128)):
  nll = -mean_b(scores[b] - log_z[b]) / 100

  scores[b] = sum_s em[b,s,tag_s]                       (device, PE diag-gather)
            + T[BOS,tag_0] + sum_s T[tag_{s-1},tag_s] + T[tag_last,EOS]
                                                        (host fp64: tags+T only)
  log_z[b]  = forward algorithm over the 128 real labels (BOS/EOS rows/cols
              are exactly unreachable: exp(-10000) == 0 in fp32).

Device strategy (8 cores x 4 chains = 32 sequence chunks of 64 steps):
  * Forward recursion in the exp domain: q <- (q @ expT) * exp(em_s - K) with
    constant per-step rescale exp(-K). Each chunk starts from a uniform vector
    with W=8 warmup steps; the dense random CRF forward map contracts to ~1e-8
    within 8 steps, so each chunk's log-gain is exact. Chunk gains telescope:
      log_z = phi_end(chunk0) + sum_{others}(phi_end - phi_pre) + 2047*K.
    Chunk 0 gets the exact initial state u0 = exp(em_0 + T[BOS,:]) blended in
    via a data-driven gamma scalar.
  * The 4 chains form 2 groups of 2; each group's two chains share one
    [128,512] matmul + one fused PSUM-multiply DVE op per step, and the two
    groups ping-pong to hide PE<->DVE latency.
  * Gold-path emission score via PE diag-accumulate: one-hot masks M_s (fp8)
    as stationary weights against the same emission stream the scan reads:
      dacc_h[b',b] += sum_l M_s[l,b'] em_s[l,b]   (per batch half)
    whose diagonal is the per-batch emission score sum.

The program is fully SPMD: per-core differences ride in the input data
(zero-padded warmup slices, gamma blend scalars, BOS bias vectors, final
functional vectors).
"""
import sys, os

for _p in ("/opt/trn_rl_repo",):
    if _p not in sys.path and os.path.isdir(_p):
        sys.path.insert(0, _p)

import numpy as np
import ml_dtypes

B, S, NL = 256, 2048, 128
NB, BOS, EOS = 130, 128, 129
NCORES = 8
NCHAIN = 4             # chains per core (2 groups of 2)
NGRP = 2
CSTEP = S // (NCORES * NCHAIN)   # 64 real steps per chain
W = 8                  # warmup slots per chain
TILES = W + CSTEP      # 72 slots per chain
CHUNK = 8              # slots per DMA/exp chunk
NCH = TILES // CHUNK   # 9 chunks per chain
GCB = CHUNK * 2 * B    # columns per (chunk, group) block: t8-major, chain, b
F8 = ml_dtypes.float8_e4m3
BF16 = ml_dtypes.bfloat16

_prog_cache = {}


def _estimate_K(em, T):
    """Mean per-step log-growth of the forward recursion (host, tiny presim)."""
    expT = np.exp(T[:NL, :NL].astype(np.float64))
    nb = 4
    v = np.exp(T[BOS, :NL].astype(np.float64)[None, :] + em[:nb, 0, :].astype(np.float64))
    g = []
    for s in range(1, 33):
        v = (v @ expT) * np.exp(em[:nb, s, :].astype(np.float64))
        n = v.sum(axis=1)
        g.append(np.log(n))
        v /= n[:, None]
    g = np.array(g[8:])  # skip mixing transient
    return float(g.mean())


def _host_prep(emissions, tags, transitions):
    em = np.asarray(emissions, np.float32)
    tg = np.asarray(tags, np.int64)
    T = np.asarray(transitions, np.float32)

    K = _estimate_K(em, T)
    expT_bf = np.exp(T[:NL, :NL]).astype(BF16)            # [prev, cur]
    teos_bf = np.exp(T[:NL, EOS]).astype(BF16)

    em_t = np.ascontiguousarray(em.transpose(1, 2, 0)).astype(F8)     # [S, 128, B]
    M = np.zeros((S, NL, B), F8)
    M[np.arange(S)[:, None], tg.T, np.arange(B)[None, :]] = 1.0

    # transition part of the gold score: host fp64, touches only tags + T
    T64 = T.astype(np.float64)
    trans_sc = (T64[BOS, tg[:, 0]]
                + T64[tg[:, :-1], tg[:, 1:]].sum(axis=1)
                + T64[tg[:, -1], EOS])                                 # [B]

    in_maps = []
    for k in range(NCORES):
        # sa/sb column layout per chunk: [g, t8, ch%2, b]
        sa = np.zeros((NCH, CHUNK, NCHAIN, NL, B), F8)   # [c, t8, ch, l, b]
        sb = np.zeros((NCH, CHUNK, NCHAIN, NL, B), F8)
        tbos = np.full((NL, NCHAIN), -10000.0, np.float32)
        gam = np.ones((NL, NCHAIN), np.float32)
        for ch in range(NCHAIN):
            s0 = CSTEP * (NCHAIN * k + ch)
            lo = s0 - W
            for j in range(TILES):
                s = lo + j
                c, t8 = j // CHUNK, j % CHUNK
                if s >= 0:
                    sa[c, t8, ch] = em_t[s]
                if j >= W:
                    sb[c, t8, ch] = M[s]
            if k == 0 and ch == 0:
                tbos[:, 0] = T[BOS, :NL]
                gam[:, 0] = 0.0
        # reorder chains into (group, ch%2) column order: ch = 2g + chp
        sa = sa.reshape(NCH, CHUNK, NGRP, 2, NL, B).transpose(0, 2, 1, 3, 4, 5)
        sb = sb.reshape(NCH, CHUNK, NGRP, 2, NL, B).transpose(0, 2, 1, 3, 4, 5)
        sa = np.ascontiguousarray(sa.transpose(0, 4, 1, 2, 3, 5)).reshape(NCH, NL, NGRP * GCB)
        sb = np.ascontiguousarray(sb.transpose(0, 4, 1, 2, 3, 5)).reshape(NCH, NL, NGRP * GCB)

        cb = np.zeros((NL, 2 * NL + 1 + NCHAIN), BF16)
        cb[:, 0:NL] = expT_bf
        cb[:, NL:2 * NL] = np.eye(NL, dtype=BF16)
        cb[:, 2 * NL:2 * NL + 1] = 1.0
        for ch in range(NCHAIN):
            last = (k == NCORES - 1 and ch == NCHAIN - 1)
            cb[:, 2 * NL + 1 + ch] = teos_bf if last else np.ones(NL, BF16)
        cf = np.zeros((NL, 2 * NCHAIN), np.float32)
        cf[:, 0:NCHAIN] = tbos
        cf[:, NCHAIN:2 * NCHAIN] = gam
        in_maps.append({"sa": sa, "sb": sb, "cbf": cb, "cfp": cf})
    return in_maps, K, trans_sc


def _build_program(K):
    import contextlib
    import concourse.bass as bass
    import concourse.tile as tile
    from concourse import bacc, mybir

    dt = mybir.dt
    Alu = mybir.AluOpType
    Act = mybir.ActivationFunctionType

    nc = bacc.Bacc("TRN2", target_bir_lowering=False, debug=False, num_devices=NCORES)

    sa_d = nc.dram_tensor("sa", [NCH, NL, NGRP * GCB], dt.float8e4, kind="ExternalInput").ap()
    sb_d = nc.dram_tensor("sb", [NCH, NL, NGRP * GCB], dt.float8e4, kind="ExternalInput").ap()
    cbf_d = nc.dram_tensor("cbf", [NL, 2 * NL + 1 + NCHAIN], dt.bfloat16, kind="ExternalInput").ap()
    cfp_d = nc.dram_tensor("cfp", [NL, 2 * NCHAIN], dt.float32, kind="ExternalInput").ap()

    # per chain: [pre | end] each [1, B]
    phis_d = nc.dram_tensor("phis", [1, NCHAIN * 2 * B], dt.float32, kind="ExternalOutput").ap()
    etpart_d = nc.dram_tensor("etpart", [NL, 2], dt.float32, kind="ExternalOutput").ap()

    with tile.TileContext(nc) as tc:
        with contextlib.ExitStack() as ctx:
            const = ctx.enter_context(tc.tile_pool(name="const", bufs=1))
            emring = ctx.enter_context(tc.tile_pool(name="emring", bufs=3))
            exring = ctx.enter_context(tc.tile_pool(name="exring", bufs=2))
            dring = ctx.enter_context(tc.tile_pool(name="dring", bufs=3))
            ps = ctx.enter_context(tc.tile_pool(name="ps", bufs=1, space="PSUM"))

            cbf = const.tile([NL, 2 * NL + 1 + NCHAIN], dt.bfloat16)
            nc.sync.dma_start(cbf[:], cbf_d[:])
            cfp = const.tile([NL, 2 * NCHAIN], dt.float32)
            nc.sync.dma_start(cfp[:], cfp_d[:])
            expT = cbf[:, 0:NL]
            ident = cbf[:, NL:2 * NL]
            fones = cbf[:, 2 * NL:2 * NL + 1]
            fvec = cbf[:, 2 * NL + 1:2 * NL + 1 + NCHAIN]
            tbos = cfp[:, 0:NCHAIN]
            gam = cfp[:, NCHAIN:2 * NCHAIN]
            negK = const.tile([NL, 1], dt.float32)
            nc.vector.memset(negK[:], -K)

            qg = []
            for g in range(NGRP):
                q = const.tile([NL, 2 * B], dt.bfloat16, name=f"q{g}")
                nc.vector.memset(q[:], 1.0)
                qg.append(q)
            us = [const.tile([NL, B], dt.bfloat16, name=f"u{ch}") for ch in range(NCHAIN)]

            psg = [ps.tile([NL, 2 * B], dt.float32, name=f"psg{g}") for g in range(NGRP)]
            dacc = [ps.tile([NL, NL], dt.float32, name=f"dacc{h}") for h in range(2)]
            phi_pre = ps.tile([1, NCHAIN * B], dt.float32)
            phi_end = ps.tile([1, NCHAIN * B], dt.float32)

            n_dacc = NCH * CHUNK * NCHAIN  # per half
            i_dacc = 0
            for c in range(NCH):
                sa_t = emring.tile([NL, NGRP * GCB], dt.float8e4, name=f"sa{c}", tag="em")
                nc.sync.dma_start(sa_t[:], sa_d[c])
                exc = exring.tile([NL, NGRP * GCB], dt.bfloat16, name=f"ex{c}", tag="ex")
                for g in range(NGRP):
                    nc.scalar.activation(exc[:, g * GCB:(g + 1) * GCB],
                                         sa_t[:, g * GCB:(g + 1) * GCB],
                                         Act.Exp, bias=negK[:], scale=1.0)
                if c == W // CHUNK:
                    # first real step of each chain (slot t8=0 of chunk 1)
                    for ch in range(NCHAIN):
                        g, chp = ch // 2, ch % 2
                        off = g * GCB + chp * B
                        nc.scalar.activation(us[ch][:], sa_t[:, off:off + B],
                                             Act.Exp, bias=tbos[:, ch:ch + 1], scale=1.0)
                sb_t = dring.tile([NL, NGRP * GCB], dt.float8e4, name=f"sb{c}", tag="d")
                nc.sync.dma_start(sb_t[:], sb_d[c])

                for t8 in range(CHUNK):
                    t = c * CHUNK + t8
                    for g in range(NGRP):
                        q, p = qg[g], psg[g]
                        if t == W:
                            for chp in range(2):
                                ch = 2 * g + chp
                                nc.tensor.matmul(phi_pre[:, ch * B:(ch + 1) * B], fones[:],
                                                 q[:, chp * B:(chp + 1) * B],
                                                 start=True, stop=True)
                        nc.tensor.matmul(p[:], expT[:], q[:], start=True, stop=True)
                        nc.vector.tensor_tensor(
                            q[:], p[:], exc[:, g * GCB + t8 * 2 * B: g * GCB + (t8 + 1) * 2 * B],
                            Alu.mult)
                        if t == W:
                            for chp in range(2):
                                ch = 2 * g + chp
                                qh = q[:, chp * B:(chp + 1) * B]
                                nc.vector.scalar_tensor_tensor(qh, qh, gam[:, ch:ch + 1],
                                                               us[ch][:], Alu.mult, Alu.add)
                    # gold-score diag accumulate off the same emission stream
                    for g in range(NGRP):
                        for chp in range(2):
                            base = g * GCB + t8 * 2 * B + chp * B
                            for h in range(2):
                                nc.tensor.matmul(
                                    dacc[h][:],
                                    sb_t[:, base + h * NL: base + (h + 1) * NL],
                                    sa_t[:, base + h * NL: base + (h + 1) * NL],
                                    start=(i_dacc == 0), stop=(i_dacc == n_dacc - 1))
                            i_dacc += 1

            for ch in range(NCHAIN):
                g, chp = ch // 2, ch % 2
                nc.tensor.matmul(phi_end[:, ch * B:(ch + 1) * B], fvec[:, ch:ch + 1],
                                 qg[g][:, chp * B:(chp + 1) * B], start=True, stop=True)

            phi_sb = const.tile([1, NCHAIN * 2 * B], dt.float32)
            nc.scalar.copy(phi_sb[:, 0:NCHAIN * B], phi_pre[:])
            nc.scalar.copy(phi_sb[:, NCHAIN * B:2 * NCHAIN * B], phi_end[:])
            nc.sync.dma_start(phis_d[:], phi_sb[:])

            escr = const.tile([NL, NL], dt.bfloat16)
            etp = const.tile([NL, 2], dt.float32)
            for h in range(2):
                nc.vector.scalar_tensor_tensor(escr[:], dacc[h][:], 1.0, ident[:],
                                               Alu.mult, Alu.mult, accum_out=etp[:, h:h + 1])
            nc.sync.dma_start(etpart_d[:], etp[:])

    nc.compile()
    return nc


def run(emissions, tags, transitions, trace=False, trace_cores=None):
    from concourse.bass_utils import run_bass_kernel_spmd
    in_maps, K, trans_sc = _host_prep(emissions, tags, transitions)
    key = f"{K:.9f}"
    if key not in _prog_cache:
        _prog_cache[key] = _build_program(K)
    nc = _prog_cache[key]
    r = run_bass_kernel_spmd(nc, in_maps, list(range(NCORES)), trace=trace,
                             trace_cores=trace_cores)

    # phis per core: [pre0..pre3 | end0..end3] each [B]
    raw = np.stack([r.results[k]["phis"].reshape(2 * NCHAIN, B) for k in range(NCORES)])
    raw = np.log(raw.astype(np.float64))
    pre = raw[:, 0:NCHAIN].reshape(NCORES * NCHAIN, B)
    end = raw[:, NCHAIN:2 * NCHAIN].reshape(NCORES * NCHAIN, B)
    log_z = end[0] + end[1:].sum(0) - pre[1:].sum(0) + 2047.0 * K

    etp = np.stack([r.results[k]["etpart"] for k in range(NCORES)]).sum(0)  # [128, 2]
    em_sc = etp.transpose(1, 0).reshape(2 * NL).astype(np.float64)          # [B]
    scores = em_sc + trans_sc
    nll = -np.mean(scores - log_z) / 100.0
    return np.float32(nll), r


def kernel(emissions, tags, transitions):
    out, _ = run(emissions, tags, transitions, trace=False)
    return out
